# revision 29
# baseline (speedup 1.0000x reference)
"""Trainium2 Bass kernel for nn_LocalAttentionParallel.

Reference computation (per batch element b):
    qkv = x @ W_qkv + b_qkv ; q,k,v = split(qkv)
    scores = (q @ k^T) * scale, masked to causal sliding window of width 128
    out = LayerNorm(scores @ v) * ln_w + ln_b          (no softmax!)

Sharding: data-parallel over batch B=8 across 8 NeuronCores (1 element each).
Weights replicated. ln_w/ln_b affine applied on host (free; device returns the
normalized tensor).

Device algorithm per core (T=2048, D=768, span=128):
  - 16 key blocks of 128 tokens. Query block t needs keys from blocks t-1, t.
  - All GEMMs in bf16 (1 cycle/row on the PE, same as fp32r, but half the
    HBM/SBUF traffic so weights arrive before the PE needs them). PSUM
    accumulation is fp32; LN statistics are fp32.
  - q^T, k^T (embedding on partitions) come straight out of the projection
    matmuls; v in natural layout. A 769th column of W_v (host-added row sums)
    makes the PE produce row-sums of the attention output for the LN mean.
  - Attention is software-pipelined: S^T(kb+1) is computed while S@V(kb)
    runs, hiding the mask-multiply latency between them.
"""

import numpy as np
import ml_dtypes

import concourse.bass as bass
import concourse.mybir as mybir
import concourse.tile as tile
from concourse import bacc
from concourse import bass_utils

F32 = mybir.dt.float32
BF16 = mybir.dt.bfloat16
AF = mybir.ActivationFunctionType
ALU = mybir.AluOpType

B, T, D = 8, 2048, 768
SPAN = 128
NCHK = 6          # contraction chunks of 128 over D
NB = 16           # 128-token blocks
NM = 4            # 512-token projection chunks
TPAD = T + 128    # q^T padded so the last S^T matmul can read a full 256 span
LN_EPS = 1e-5
SCALE = 1.0 / np.sqrt(D * SPAN)

MMDT = BF16
NPDT = ml_dtypes.bfloat16

_cache = {}


def _mm(nc, out, lhsT, rhs, **kw):
    nc.tensor.matmul(out, lhsT, rhs, **kw)


def _build():
    nc = bacc.Bacc("TRN2", target_bir_lowering=False, debug=False,
                   enable_asserts=False, num_devices=8)
    xT = nc.dram_tensor("xT", [D, T], MMDT, kind="ExternalInput").ap()
    W = nc.dram_tensor("W", [12, 128, NCHK * 128], MMDT, kind="ExternalInput").ap()
    WVA = nc.dram_tensor("WVA", [NCHK, 128, D + 4], MMDT, kind="ExternalInput").ap()
    BQK = nc.dram_tensor("BQK", [128, 12], F32, kind="ExternalInput").ap()
    BV = nc.dram_tensor("BV", [128, D + 4], F32, kind="ExternalInput").ap()
    MSK = nc.dram_tensor("MSK", [128, 256], F32, kind="ExternalInput").ap()
    OUT = nc.dram_tensor("out", [T, D], MMDT, kind="ExternalOutput").ap()

    with tile.TileContext(nc) as tc:
        xT_r = xT.rearrange("(c p) t -> p c t", p=128)
        with tc.tile_pool(name="const", bufs=1) as cp, \
             tc.tile_pool(name="xt", bufs=2) as xp, \
             tc.tile_pool(name="kt", bufs=2) as kp, \
             tc.tile_pool(name="vt", bufs=6) as vp, \
             tc.tile_pool(name="st", bufs=3) as stp, \
             tc.tile_pool(name="outp", bufs=3) as outp, \
             tc.tile_pool(name="scr", bufs=2) as scrp, \
             tc.tile_pool(name="stat", bufs=24) as sp, \
             tc.tile_pool(name="pp", bufs=2, space="PSUM") as pp, \
             tc.tile_pool(name="sps", bufs=2, space="PSUM") as sps, \
             tc.tile_pool(name="ops", bufs=4, space="PSUM") as ops:

            # ---- prefetch first x chunk (sync queue; weights go on scalar) ----
            xt0 = xp.tile([128, NCHK, 512], MMDT, tag="xt", name="xt0")
            for c in range(NCHK):
                nc.sync.dma_start(xt0[:, c, :], xT_r[:, c, 0:512])
            # ---- weights + consts in first-use order ----
            wqk = []
            for e in range(12):
                w = cp.tile([128, NCHK, 128], MMDT, tag=f"wqk{e}", name="wqk")
                wqk.append(w)
            nc.scalar.dma_start(wqk[0][:], W[0].rearrange("p (c q) -> p c q", c=NCHK))
            bqk = cp.tile([128, 12], F32, tag="bqk")
            nc.scalar.dma_start(bqk[:], BQK)
            for e in range(1, 6):
                nc.scalar.dma_start(
                    wqk[e][:], W[e].rearrange("p (c q) -> p c q", c=NCHK))
            msk = cp.tile([128, 256], F32, tag="msk")
            nc.scalar.dma_start(msk[:], MSK)
            for e in range(6, 12):
                nc.scalar.dma_start(
                    wqk[e][:], W[e].rearrange("p (c q) -> p c q", c=NCHK))
            bv = cp.tile([128, D + 4], F32, tag="bv")
            nc.scalar.dma_start(bv[:], BV)
            wv = []
            for c in range(NCHK):
                w = cp.tile([128, D + 4], MMDT, tag=f"wv{c}")
                nc.scalar.dma_start(w[:], WVA[c])
                wv.append(w)
            eps = cp.tile([128, 1], F32, tag="eps")
            nc.vector.memset(eps[:], LN_EPS)

            # persistent q^T, padded with zeros past T (no input dependency!)
            qt = []
            for c in range(NCHK):
                q = cp.tile([128, TPAD], MMDT, tag=f"qt{c}")
                nc.vector.memset(q[:, T:TPAD], 0.0)
                qt.append(q)

            kt_tiles = {}
            v_tiles = {}
            o_tiles = {}
            st_tiles = {}

            def proj_q(m, xt):
                """q^T slices for tokens [512m, 512m+512): e on partitions."""
                for e in range(6):
                    ps = pp.tile([128, 512], F32, tag="proj")
                    for c in range(NCHK):
                        _mm(nc, ps[:], wqk[e][:, c, :],
                            xt[:, c, :], start=(c == 0), stop=(c == NCHK - 1))
                    nc.vector.tensor_scalar_add(
                        qt[e][:, 512 * m:512 * (m + 1)], ps[:], bqk[:, e:e + 1])

            def proj_k(m, xt):
                # k^T: one tile per 512-chunk [128, 6, 512]
                kt_m = kp.tile([128, NCHK, 512], MMDT, tag="kt", name="ktm")
                kt_tiles[m] = kt_m
                for e in range(6):
                    ps = pp.tile([128, 512], F32, tag="proj")
                    for c in range(NCHK):
                        _mm(nc, ps[:], wqk[6 + e][:, c, :],
                            xt[:, c, :], start=(c == 0), stop=(c == NCHK - 1))
                    nc.scalar.activation(kt_m[:, e, :], ps[:], AF.Identity,
                                         bias=bqk[:, 6 + e:7 + e])

            def proj_v(m, xt):
                # v natural (+ aug row-sum col), per 128-token quarter
                for h in range(4):
                    psA = pp.tile([128, 384], F32, tag="proj")
                    psB = pp.tile([128, 388], F32, tag="proj")
                    for c in range(NCHK):
                        _mm(nc, psA[:], xt[:, c, 128 * h:128 * (h + 1)],
                            wv[c][:, 0:384], start=(c == 0), stop=(c == NCHK - 1))
                    for c in range(NCHK):
                        _mm(nc, psB[:], xt[:, c, 128 * h:128 * (h + 1)],
                            wv[c][:, 384:772], start=(c == 0), stop=(c == NCHK - 1))
                    vt = vp.tile([128, D + 4], MMDT, tag="v")
                    nc.vector.tensor_tensor(vt[:, 0:384], psA[:], bv[:, 0:384],
                                            op=ALU.add)
                    nc.vector.tensor_tensor(vt[:, 384:772], psB[:], bv[:, 384:772],
                                            op=ALU.add)
                    v_tiles[4 * m + h] = vt

            ln_state = {}

            def ln_stats(kb, early_copy=False):
                """PSUM-reading stats: squares (Act) + mean/bias (DVE).
                early_copy also snapshots oa/ob to SBUF bf16 so the o PSUM
                banks free promptly and the norms run from SBUF."""
                oa, ob = o_tiles.pop(kb)
                neg_mu = sp.tile([128, 1], F32, tag="stat")
                nc.vector.tensor_scalar_mul(neg_mu[:], ob[:, 384:385], -1.0 / D)
                # bias_e = -mu^2  (eps is folded into sum2e below)
                bias_e = sp.tile([128, 1], F32, tag="stat")
                nc.vector.tensor_scalar(bias_e[:], neg_mu[:], neg_mu[:], -1.0,
                                        op0=ALU.mult, op1=ALU.mult)
                ssqa = sp.tile([128, 1], F32, tag="stat")
                ssqb = sp.tile([128, 1], F32, tag="stat")
                sum2e = sp.tile([128, 1], F32, tag="stat")
                if early_copy:
                    # snapshot to SBUF (DVE), square-reduce on the idle Pool
                    # engine, leaving Act only sqrt+norm in the endgame tail
                    ca = scrp.tile([128, 384], MMDT, tag="ca", bufs=4)
                    nc.vector.tensor_scalar_mul(ca[:], oa[:, 0:384], 1.0)
                    cb = scrp.tile([128, 384], MMDT, tag="cb", bufs=4)
                    nc.vector.tensor_scalar_mul(cb[:], ob[:, 0:384], 1.0)
                    scr = scrp.tile([128, 384], F32, tag="scr")
                    nc.scalar.activation(scr[:], ca[:], AF.Square,
                                         accum_out=ssqa[:])
                    scr2 = scrp.tile([128, 384], F32, tag="scr")
                    nc.scalar.activation(scr2[:], cb[:], AF.Square,
                                         accum_out=ssqb[:])
                    nc.vector.tensor_scalar(sum2e[:], ssqa[:], ssqb[:],
                                            float(LN_EPS * D),
                                            op0=ALU.add, op1=ALU.add)
                    ln_state[kb] = (ca, cb, neg_mu, bias_e, sum2e)
                else:
                    scr = scrp.tile([128, 384], F32, tag="scr")
                    nc.scalar.activation(scr[:], oa[:, 0:384], AF.Square,
                                         accum_out=ssqa[:])
                    scr2 = scrp.tile([128, 384], F32, tag="scr")
                    nc.scalar.activation(scr2[:], ob[:, 0:384], AF.Square,
                                         accum_out=ssqb[:])
                    nc.vector.tensor_scalar(sum2e[:], ssqa[:], ssqb[:],
                                            float(LN_EPS * D),
                                            op0=ALU.add, op1=ALU.add)
                    ln_state[kb] = (oa, ob, neg_mu, bias_e, sum2e)

            def ln_norm(kb):
                """sqrt -> rstd -> concurrent normalizes (DVE a, Act b)."""
                sa, sb, neg_mu, bias_e, sum2e = ln_state.pop(kb)
                std = sp.tile([128, 1], F32, tag="stat")
                nc.scalar.activation(std[:], sum2e[:], AF.Sqrt, bias=bias_e[:],
                                     scale=1.0 / D)
                rstd = sp.tile([128, 1], F32, tag="stat")
                nc.vector.reciprocal(rstd[:], std[:])
                bias_b = sp.tile([128, 1], F32, tag="stat")
                nc.gpsimd.tensor_scalar_mul(bias_b[:], neg_mu[:], rstd[:])
                osb = outp.tile([128, D], MMDT, tag="out")
                nc.vector.tensor_scalar(osb[:, 0:384], sa[:, 0:384],
                                        neg_mu[:], rstd[:],
                                        op0=ALU.add, op1=ALU.mult)
                nc.scalar.activation(osb[:, 384:768], sb[:, 0:384], AF.Identity,
                                     bias=bias_b[:], scale=rstd[:])
                if kb == NB - 1:
                    # last block: split the store so the first half goes out
                    # as soon as the DVE-normalized half is ready
                    nc.sync.dma_start(OUT[128 * kb:128 * (kb + 1), 0:384],
                                      osb[:, 0:384])
                    nc.sync.dma_start(OUT[128 * kb:128 * (kb + 1), 384:768],
                                      osb[:, 384:768])
                else:
                    nc.sync.dma_start(OUT[128 * kb:128 * (kb + 1), :], osb[:])

            def ln_store(kb):
                ln_stats(kb)
                ln_norm(kb)

            def score(kb):
                # S^T for key block kb vs queries [128kb, 128kb+256)
                # (last block: only its own 128 queries exist)
                w = 128 if kb == NB - 1 else 256
                st_ps = sps.tile([128, 256], F32, tag="st")
                ktile = kt_tiles[kb // 4]
                koff = 128 * (kb % 4)
                for c in range(NCHK):
                    _mm(nc, st_ps[:, 0:w], ktile[:, c, koff:koff + 128],
                        qt[c][:, 128 * kb:128 * kb + w],
                        start=(c == 0), stop=(c == NCHK - 1))
                st_sb = stp.tile([128, 256], MMDT, tag="stsb")
                nc.vector.tensor_tensor(st_sb[:, 0:w], st_ps[:, 0:w],
                                        msk[:, 0:w], op=ALU.mult)
                st_tiles[kb] = st_sb

            def av(kb):
                st_sb = st_tiles.pop(kb)
                vt = v_tiles.pop(kb)
                if kb == 0:
                    o_tiles[0] = (ops.tile([128, 384], F32, tag="o", name="o0a"),
                                  ops.tile([128, 388], F32, tag="o", name="o0b"))
                oa, ob = o_tiles[kb]
                _mm(nc, oa[:], st_sb[:, 0:128], vt[:, 0:384],
                    start=(kb == 0), stop=True, skip_group_check=True)
                _mm(nc, ob[:], st_sb[:, 0:128], vt[:, 384:772],
                    start=(kb == 0), stop=True, skip_group_check=True)
                if kb < NB - 1:
                    # endgame (no proj left): borrow the idle pp banks for
                    # o(14) so the last av blocks never wait on ln ladders
                    pool = pp if kb + 1 == 14 else ops
                    tag = "proj" if pool is pp else "o"
                    na = pool.tile([128, 384], F32, tag=tag, name="ona")
                    nb_ = pool.tile([128, 388], F32, tag=tag, name="onb")
                    o_tiles[kb + 1] = (na, nb_)
                    _mm(nc, na[:], st_sb[:, 128:256], vt[:, 0:384],
                        start=True, stop=False, skip_group_check=True)
                    _mm(nc, nb_[:], st_sb[:, 128:256], vt[:, 384:772],
                        start=True, stop=False, skip_group_check=True)

            proj_q(0, xt0)
            proj_k(0, xt0)
            proj_v(0, xt0)
            score(0)
            xts = {0: xt0}

            def prefetch_x(mm):
                xt = xp.tile([128, NCHK, 512], MMDT, tag="xt")
                for c in range(NCHK):
                    nc.sync.dma_start(
                        xt[:, c, :], xT_r[:, c, 512 * mm:512 * (mm + 1)])
                xts[mm] = xt

            prefetch_x(1)
            for m in range(NM - 1):
                # proj sub-phases interleave with attention so the PE covers
                # the LN ladder; score(4m+3) needs proj_q(m+1)'s qt columns,
                # score(4m+4) needs proj_k(m+1)'s k^T. Each ln ladder is
                # emitted AFTER the following proj phase so the proj PSUM
                # drains sit ahead of it in the engine queues.
                score(4 * m + 1)
                av(4 * m)
                proj_q(m + 1, xts[m + 1])
                if m + 2 < NM:
                    prefetch_x(m + 2)
                ln_store(4 * m)
                score(4 * m + 2)
                av(4 * m + 1)
                proj_k(m + 1, xts[m + 1])
                ln_store(4 * m + 1)
                score(4 * m + 3)
                av(4 * m + 2)
                proj_v(m + 1, xts[m + 1])
                ln_store(4 * m + 2)
                score(4 * m + 4)
                av(4 * m + 3)
                ln_store(4 * m + 3)
            # endgame (m=3): no proj left to hide ladders behind. Pipeline
            # stats/norm two-deep so the scalar queue never head-of-line
            # blocks at sqrt, and the last av mms run back-to-back.
            score(13)
            av(12)
            ln_stats(12, early_copy=True)
            score(14)
            av(13)           # o(14) -> pp, no ladder wait
            ln_stats(13, early_copy=True)
            ln_norm(12)
            score(15)
            av(14)           # o(15) -> ops slots freed by copies(12)
            ln_stats(14, early_copy=True)
            ln_norm(13)
            av(15)
            ln_stats(15, early_copy=True)
            ln_norm(14)
            ln_norm(15)

    nc.compile()
    return nc


def _prepare_common(W_qkv, b_qkv):
    Wfull = np.ascontiguousarray(W_qkv, dtype=np.float32)
    W = np.empty((12, 128, NCHK * 128), dtype=np.float32)
    for e in range(12):
        for c in range(NCHK):
            W[e, :, 128 * c:128 * (c + 1)] = \
                Wfull[128 * c:128 * (c + 1), 128 * e:128 * (e + 1)]
    wv = Wfull[:, 1536:2304]
    WVA = np.zeros((NCHK, 128, D + 4), dtype=np.float32)
    for c in range(NCHK):
        blk = wv[128 * c:128 * (c + 1)]
        WVA[c, :, 0:D] = blk
        WVA[c, :, D] = blk.sum(axis=1)
    BQK = np.ascontiguousarray(
        b_qkv[0:1536].reshape(12, 128).T, dtype=np.float32)
    bva = np.zeros(D + 4, dtype=np.float32)
    bva[0:D] = b_qkv[1536:2304]
    bva[D] = b_qkv[1536:2304].sum()
    BV = np.ascontiguousarray(np.broadcast_to(bva, (128, D + 4)))
    j = np.arange(128)[:, None]
    i = np.arange(256)[None, :]
    MSK = np.where((i - j >= 0) & (i - j < SPAN), SCALE, 0.0).astype(np.float32)
    return W.astype(NPDT), WVA.astype(NPDT), BQK, BV, MSK


def run(inputs, trace=False):
    x = np.asarray(inputs["x"], dtype=np.float32)
    W_qkv = np.asarray(inputs["W_qkv"], dtype=np.float32)
    b_qkv = np.asarray(inputs["b_qkv"], dtype=np.float32)
    if "nc" not in _cache:
        _cache["nc"] = _build()
    nc = _cache["nc"]
    W, WVA, BQK, BV, MSK = _prepare_common(W_qkv, b_qkv)
    xT = np.ascontiguousarray(x.transpose(0, 2, 1)).astype(NPDT)  # [B, D, T]
    in_maps = [
        {"xT": xT[b], "W": W, "WVA": WVA, "BQK": BQK, "BV": BV, "MSK": MSK}
        for b in range(B)
    ]
    res = bass_utils.run_bass_kernel_spmd(
        nc, in_maps, core_ids=list(range(B)), trace=trace)
    return res


def kernel(x, W_qkv, b_qkv, ln_w, ln_b):
    res = run({"x": x, "W_qkv": W_qkv, "b_qkv": b_qkv})
    out = np.stack([np.asarray(res.results[b]["out"], dtype=np.float32)
                    for b in range(B)])
    ln_w = np.asarray(ln_w, dtype=np.float32)
    ln_b = np.asarray(ln_b, dtype=np.float32)
    if not (np.all(ln_w == 1.0) and np.all(ln_b == 0.0)):
        out = out * ln_w + ln_b
    return out


# revision 32
# speedup vs baseline: 1.0352x; 1.0352x over previous
"""Trainium2 Bass kernel for nn_LocalAttentionParallel.

Reference computation (per batch element b):
    qkv = x @ W_qkv + b_qkv ; q,k,v = split(qkv)
    scores = (q @ k^T) * scale, masked to causal sliding window of width 128
    out = LayerNorm(scores @ v) * ln_w + ln_b          (no softmax!)

Sharding: data-parallel over batch B=8 across 8 NeuronCores (1 element each).
Weights replicated. ln_w/ln_b affine applied on host (free; device returns the
normalized tensor).

Device algorithm per core (T=2048, D=768, span=128):
  - 16 key blocks of 128 tokens. Query block t needs keys from blocks t-1, t.
  - All GEMMs in bf16 (1 cycle/row on the PE, same as fp32r, but half the
    HBM/SBUF traffic so weights arrive before the PE needs them). PSUM
    accumulation is fp32; LN statistics are fp32.
  - q^T, k^T (embedding on partitions) come straight out of the projection
    matmuls; v in natural layout. A 769th column of W_v (host-added row sums)
    makes the PE produce row-sums of the attention output for the LN mean.
  - Attention is software-pipelined: S^T(kb+1) is computed while S@V(kb)
    runs, hiding the mask-multiply latency between them.
"""

import numpy as np
import ml_dtypes

import concourse.bass as bass
import concourse.mybir as mybir
import concourse.tile as tile
from concourse import bacc
from concourse import bass_utils

F32 = mybir.dt.float32
BF16 = mybir.dt.bfloat16
AF = mybir.ActivationFunctionType
ALU = mybir.AluOpType

B, T, D = 8, 2048, 768
SPAN = 128
NCHK = 6          # contraction chunks of 128 over D
NB = 16           # 128-token blocks
NM = 4            # 512-token projection chunks
TPAD = T + 128    # q^T padded so the last S^T matmul can read a full 256 span
LN_EPS = 1e-5
SCALE = 1.0 / np.sqrt(D * SPAN)

MMDT = BF16
NPDT = ml_dtypes.bfloat16

_cache = {}


def _mm(nc, out, lhsT, rhs, **kw):
    nc.tensor.matmul(out, lhsT, rhs, **kw)


def _build():
    nc = bacc.Bacc("TRN2", target_bir_lowering=False, debug=False,
                   enable_asserts=False, num_devices=8)
    xT = nc.dram_tensor("xT", [D, T], MMDT, kind="ExternalInput").ap()
    W = nc.dram_tensor("W", [12, 128, NCHK * 128], MMDT, kind="ExternalInput").ap()
    WVA = nc.dram_tensor("WVA", [NCHK, 128, D + 4], MMDT, kind="ExternalInput").ap()
    BQK = nc.dram_tensor("BQK", [128, 12], F32, kind="ExternalInput").ap()
    BV = nc.dram_tensor("BV", [128, D + 4], F32, kind="ExternalInput").ap()
    MSK = nc.dram_tensor("MSK", [128, 256], F32, kind="ExternalInput").ap()
    OUT = nc.dram_tensor("out", [T, D], MMDT, kind="ExternalOutput").ap()

    with tile.TileContext(nc) as tc:
        xT_r = xT.rearrange("(c p) t -> p c t", p=128)
        with tc.tile_pool(name="const", bufs=1) as cp, \
             tc.tile_pool(name="xt", bufs=2) as xp, \
             tc.tile_pool(name="kt", bufs=2) as kp, \
             tc.tile_pool(name="vt", bufs=6) as vp, \
             tc.tile_pool(name="st", bufs=3) as stp, \
             tc.tile_pool(name="outp", bufs=3) as outp, \
             tc.tile_pool(name="scr", bufs=2) as scrp, \
             tc.tile_pool(name="stat", bufs=24) as sp, \
             tc.tile_pool(name="pp", bufs=2, space="PSUM") as pp, \
             tc.tile_pool(name="sps", bufs=2, space="PSUM") as sps, \
             tc.tile_pool(name="ops", bufs=4, space="PSUM") as ops:

            # ---- prefetch first x chunk (sync queue; weights go on scalar) ----
            xt0 = xp.tile([128, NCHK, 512], MMDT, tag="xt", name="xt0")
            for c in range(NCHK):
                nc.sync.dma_start(xt0[:, c, :], xT_r[:, c, 0:512])
            # ---- weights + consts in first-use order ----
            wqk = []
            for e in range(12):
                w = cp.tile([128, NCHK, 128], MMDT, tag=f"wqk{e}", name="wqk")
                wqk.append(w)
            nc.scalar.dma_start(wqk[0][:], W[0].rearrange("p (c q) -> p c q", c=NCHK))
            bqk = cp.tile([128, 12], F32, tag="bqk")
            nc.scalar.dma_start(bqk[:], BQK)
            msk = cp.tile([128, 256], F32, tag="msk")
            nc.scalar.dma_start(msk[:], MSK)
            for e in range(1, 12):
                nc.scalar.dma_start(
                    wqk[e][:], W[e].rearrange("p (c q) -> p c q", c=NCHK))
            bv = cp.tile([128, D + 4], F32, tag="bv")
            nc.scalar.dma_start(bv[:], BV)
            wv = []
            for c in range(NCHK):
                w = cp.tile([128, D + 4], MMDT, tag=f"wv{c}")
                nc.scalar.dma_start(w[:], WVA[c])
                wv.append(w)
            eps = cp.tile([128, 1], F32, tag="eps")
            nc.vector.memset(eps[:], LN_EPS)

            # persistent q^T, padded with zeros past T (no input dependency!)
            qt = []
            for c in range(NCHK):
                q = cp.tile([128, TPAD], MMDT, tag=f"qt{c}")
                nc.vector.memset(q[:, T:TPAD], 0.0)
                qt.append(q)

            kt_tiles = {}
            v_tiles = {}
            o_tiles = {}
            st_tiles = {}

            def proj_q(m, xt):
                """q^T slices for tokens [512m, 512m+512): e on partitions."""
                for e in range(6):
                    ps = pp.tile([128, 512], F32, tag="proj")
                    for c in range(NCHK):
                        _mm(nc, ps[:], wqk[e][:, c, :],
                            xt[:, c, :], start=(c == 0), stop=(c == NCHK - 1))
                    nc.vector.tensor_scalar_add(
                        qt[e][:, 512 * m:512 * (m + 1)], ps[:], bqk[:, e:e + 1])

            def proj_k(m, xt):
                # k^T: one tile per 512-chunk [128, 6, 512]
                kt_m = kp.tile([128, NCHK, 512], MMDT, tag="kt", name="ktm")
                kt_tiles[m] = kt_m
                for e in range(6):
                    ps = pp.tile([128, 512], F32, tag="proj")
                    for c in range(NCHK):
                        _mm(nc, ps[:], wqk[6 + e][:, c, :],
                            xt[:, c, :], start=(c == 0), stop=(c == NCHK - 1))
                    nc.scalar.activation(kt_m[:, e, :], ps[:], AF.Identity,
                                         bias=bqk[:, 6 + e:7 + e])

            def proj_v(m, xt):
                # v natural (+ aug row-sum col), per 128-token quarter
                for h in range(4):
                    psA = pp.tile([128, 384], F32, tag="proj")
                    psB = pp.tile([128, 388], F32, tag="proj")
                    for c in range(NCHK):
                        _mm(nc, psA[:], xt[:, c, 128 * h:128 * (h + 1)],
                            wv[c][:, 0:384], start=(c == 0), stop=(c == NCHK - 1))
                    for c in range(NCHK):
                        _mm(nc, psB[:], xt[:, c, 128 * h:128 * (h + 1)],
                            wv[c][:, 384:772], start=(c == 0), stop=(c == NCHK - 1))
                    vt = vp.tile([128, D + 4], MMDT, tag="v")
                    nc.vector.tensor_tensor(vt[:, 0:384], psA[:], bv[:, 0:384],
                                            op=ALU.add)
                    nc.vector.tensor_tensor(vt[:, 384:772], psB[:], bv[:, 384:772],
                                            op=ALU.add)
                    v_tiles[4 * m + h] = vt

            ln_state = {}

            def ln_stats(kb):
                """PSUM-reading stats: squares (Act) + mean/bias (DVE)."""
                oa, ob = o_tiles.pop(kb)
                neg_mu = sp.tile([128, 1], F32, tag="stat")
                nc.vector.tensor_scalar_mul(neg_mu[:], ob[:, 384:385], -1.0 / D)
                # bias_e = -mu^2  (eps is folded into sum2e below)
                bias_e = sp.tile([128, 1], F32, tag="stat")
                nc.vector.tensor_scalar(bias_e[:], neg_mu[:], neg_mu[:], -1.0,
                                        op0=ALU.mult, op1=ALU.mult)
                ssqa = sp.tile([128, 1], F32, tag="stat")
                ssqb = sp.tile([128, 1], F32, tag="stat")
                sum2e = sp.tile([128, 1], F32, tag="stat")
                scr = scrp.tile([128, 384], F32, tag="scr")
                nc.scalar.activation(scr[:], oa[:, 0:384], AF.Square,
                                     accum_out=ssqa[:])
                scr2 = scrp.tile([128, 384], F32, tag="scr")
                nc.scalar.activation(scr2[:], ob[:, 0:384], AF.Square,
                                     accum_out=ssqb[:])
                nc.vector.tensor_scalar(sum2e[:], ssqa[:], ssqb[:],
                                        float(LN_EPS * D),
                                        op0=ALU.add, op1=ALU.add)
                ln_state[kb] = (oa, ob, neg_mu, bias_e, sum2e)

            def ln_norm(kb):
                """sqrt -> rstd -> concurrent normalizes (DVE a, Act b)."""
                sa, sb, neg_mu, bias_e, sum2e = ln_state.pop(kb)
                std = sp.tile([128, 1], F32, tag="stat")
                nc.scalar.activation(std[:], sum2e[:], AF.Sqrt, bias=bias_e[:],
                                     scale=1.0 / D)
                rstd = sp.tile([128, 1], F32, tag="stat")
                nc.vector.reciprocal(rstd[:], std[:])
                osb = outp.tile([128, D], MMDT, tag="out")
                nc.vector.tensor_scalar(osb[:, 0:384], sa[:, 0:384],
                                        neg_mu[:], rstd[:],
                                        op0=ALU.add, op1=ALU.mult)
                nc.vector.tensor_scalar(osb[:, 384:768], sb[:, 0:384],
                                        neg_mu[:], rstd[:],
                                        op0=ALU.add, op1=ALU.mult)
                if kb == NB - 1:
                    # last block: split the store so the first half goes out
                    # as soon as the DVE-normalized half is ready
                    nc.sync.dma_start(OUT[128 * kb:128 * (kb + 1), 0:384],
                                      osb[:, 0:384])
                    nc.sync.dma_start(OUT[128 * kb:128 * (kb + 1), 384:768],
                                      osb[:, 384:768])
                else:
                    nc.sync.dma_start(OUT[128 * kb:128 * (kb + 1), :], osb[:])

            def ln_store(kb):
                ln_stats(kb)
                ln_norm(kb)

            def score(kb):
                # S^T for key block kb vs queries [128kb, 128kb+256)
                # (last block: only its own 128 queries exist)
                w = 128 if kb == NB - 1 else 256
                st_ps = sps.tile([128, 256], F32, tag="st")
                ktile = kt_tiles[kb // 4]
                koff = 128 * (kb % 4)
                for c in range(NCHK):
                    _mm(nc, st_ps[:, 0:w], ktile[:, c, koff:koff + 128],
                        qt[c][:, 128 * kb:128 * kb + w],
                        start=(c == 0), stop=(c == NCHK - 1))
                st_sb = stp.tile([128, 256], MMDT, tag="stsb")
                nc.vector.tensor_tensor(st_sb[:, 0:w], st_ps[:, 0:w],
                                        msk[:, 0:w], op=ALU.mult)
                st_tiles[kb] = st_sb

            def av(kb):
                st_sb = st_tiles.pop(kb)
                vt = v_tiles.pop(kb)
                if kb == 0:
                    o_tiles[0] = (ops.tile([128, 384], F32, tag="o", name="o0a"),
                                  ops.tile([128, 388], F32, tag="o", name="o0b"))
                oa, ob = o_tiles[kb]
                _mm(nc, oa[:], st_sb[:, 0:128], vt[:, 0:384],
                    start=(kb == 0), stop=True, skip_group_check=True)
                _mm(nc, ob[:], st_sb[:, 0:128], vt[:, 384:772],
                    start=(kb == 0), stop=True, skip_group_check=True)
                if kb < NB - 1:
                    # endgame (no proj left): borrow the idle pp banks for
                    # o(14) so the last av blocks never wait on ln ladders
                    pool = pp if kb + 1 == 14 else ops
                    tag = "proj" if pool is pp else "o"
                    na = pool.tile([128, 384], F32, tag=tag, name="ona")
                    nb_ = pool.tile([128, 388], F32, tag=tag, name="onb")
                    o_tiles[kb + 1] = (na, nb_)
                    _mm(nc, na[:], st_sb[:, 128:256], vt[:, 0:384],
                        start=True, stop=False, skip_group_check=True)
                    _mm(nc, nb_[:], st_sb[:, 128:256], vt[:, 384:772],
                        start=True, stop=False, skip_group_check=True)

            proj_q(0, xt0)
            proj_k(0, xt0)
            proj_v(0, xt0)
            score(0)
            xts = {0: xt0}

            def prefetch_x(mm):
                xt = xp.tile([128, NCHK, 512], MMDT, tag="xt")
                for c in range(NCHK):
                    nc.sync.dma_start(
                        xt[:, c, :], xT_r[:, c, 512 * mm:512 * (mm + 1)])
                xts[mm] = xt

            prefetch_x(1)
            for m in range(NM - 1):
                # proj sub-phases interleave with attention so the PE covers
                # the LN ladder; score(4m+3) needs proj_q(m+1)'s qt columns,
                # score(4m+4) needs proj_k(m+1)'s k^T. Each ln ladder is
                # emitted AFTER the following proj phase so the proj PSUM
                # drains sit ahead of it in the engine queues.
                score(4 * m + 1)
                av(4 * m)
                proj_q(m + 1, xts[m + 1])
                if m + 2 < NM:
                    prefetch_x(m + 2)
                ln_store(4 * m)
                score(4 * m + 2)
                av(4 * m + 1)
                proj_k(m + 1, xts[m + 1])
                ln_store(4 * m + 1)
                score(4 * m + 3)
                av(4 * m + 2)
                proj_v(m + 1, xts[m + 1])
                ln_store(4 * m + 2)
                score(4 * m + 4)
                av(4 * m + 3)
                ln_store(4 * m + 3)
            # endgame (m=3): no proj left to hide ladders behind. Pipeline
            # stats/norm two-deep so the scalar queue never head-of-line
            # blocks at sqrt, and the last av mms run back-to-back.
            score(13)
            av(12)
            ln_stats(12)
            score(14)
            av(13)           # o(14) -> pp, no ladder wait
            ln_stats(13)
            ln_norm(12)
            score(15)
            av(14)           # o(15) -> ops slots freed by copies(12)
            ln_stats(14)
            ln_norm(13)
            av(15)
            ln_stats(15)
            ln_norm(14)
            ln_norm(15)

    nc.compile()
    return nc


def _prepare_common(W_qkv, b_qkv):
    Wfull = np.ascontiguousarray(W_qkv, dtype=np.float32)
    W = np.empty((12, 128, NCHK * 128), dtype=np.float32)
    for e in range(12):
        for c in range(NCHK):
            W[e, :, 128 * c:128 * (c + 1)] = \
                Wfull[128 * c:128 * (c + 1), 128 * e:128 * (e + 1)]
    wv = Wfull[:, 1536:2304]
    WVA = np.zeros((NCHK, 128, D + 4), dtype=np.float32)
    for c in range(NCHK):
        blk = wv[128 * c:128 * (c + 1)]
        WVA[c, :, 0:D] = blk
        WVA[c, :, D] = blk.sum(axis=1)
    BQK = np.ascontiguousarray(
        b_qkv[0:1536].reshape(12, 128).T, dtype=np.float32)
    bva = np.zeros(D + 4, dtype=np.float32)
    bva[0:D] = b_qkv[1536:2304]
    bva[D] = b_qkv[1536:2304].sum()
    BV = np.ascontiguousarray(np.broadcast_to(bva, (128, D + 4)))
    j = np.arange(128)[:, None]
    i = np.arange(256)[None, :]
    MSK = np.where((i - j >= 0) & (i - j < SPAN), SCALE, 0.0).astype(np.float32)
    return W.astype(NPDT), WVA.astype(NPDT), BQK, BV, MSK


def run(inputs, trace=False):
    x = np.asarray(inputs["x"], dtype=np.float32)
    W_qkv = np.asarray(inputs["W_qkv"], dtype=np.float32)
    b_qkv = np.asarray(inputs["b_qkv"], dtype=np.float32)
    if "nc" not in _cache:
        _cache["nc"] = _build()
    nc = _cache["nc"]
    W, WVA, BQK, BV, MSK = _prepare_common(W_qkv, b_qkv)
    xT = np.ascontiguousarray(x.transpose(0, 2, 1)).astype(NPDT)  # [B, D, T]
    in_maps = [
        {"xT": xT[b], "W": W, "WVA": WVA, "BQK": BQK, "BV": BV, "MSK": MSK}
        for b in range(B)
    ]
    res = bass_utils.run_bass_kernel_spmd(
        nc, in_maps, core_ids=list(range(B)), trace=trace)
    return res


def kernel(x, W_qkv, b_qkv, ln_w, ln_b):
    res = run({"x": x, "W_qkv": W_qkv, "b_qkv": b_qkv})
    out = np.stack([np.asarray(res.results[b]["out"], dtype=np.float32)
                    for b in range(B)])
    ln_w = np.asarray(ln_w, dtype=np.float32)
    ln_b = np.asarray(ln_b, dtype=np.float32)
    if not (np.all(ln_w == 1.0) and np.all(ln_b == 0.0)):
        out = out * ln_w + ln_b
    return out


# revision 34
# speedup vs baseline: 1.0479x; 1.0123x over previous
"""Trainium2 Bass kernel for nn_LocalAttentionParallel.

Reference computation (per batch element b):
    qkv = x @ W_qkv + b_qkv ; q,k,v = split(qkv)
    scores = (q @ k^T) * scale, masked to causal sliding window of width 128
    out = LayerNorm(scores @ v) * ln_w + ln_b          (no softmax!)

Sharding: data-parallel over batch B=8 across 8 NeuronCores (1 element each).
Weights replicated. ln_w/ln_b affine applied on host (free; device returns the
normalized tensor).

Device algorithm per core (T=2048, D=768, span=128):
  - 16 key blocks of 128 tokens. Query block t needs keys from blocks t-1, t.
  - All GEMMs in bf16 (1 cycle/row on the PE, same as fp32r, but half the
    HBM/SBUF traffic so weights arrive before the PE needs them). PSUM
    accumulation is fp32; LN statistics are fp32.
  - q^T, k^T (embedding on partitions) come straight out of the projection
    matmuls; v in natural layout. A 769th column of W_v (host-added row sums)
    makes the PE produce row-sums of the attention output for the LN mean.
  - Attention is software-pipelined: S^T(kb+1) is computed while S@V(kb)
    runs, hiding the mask-multiply latency between them.
"""

import numpy as np
import ml_dtypes

import concourse.bass as bass
import concourse.mybir as mybir
import concourse.tile as tile
from concourse import bacc
from concourse import bass_utils

F32 = mybir.dt.float32
BF16 = mybir.dt.bfloat16
AF = mybir.ActivationFunctionType
ALU = mybir.AluOpType

B, T, D = 8, 2048, 768
SPAN = 128
NCHK = 6          # contraction chunks of 128 over D
NB = 16           # 128-token blocks
NM = 4            # 512-token projection chunks
TPAD = T + 128    # q^T padded so the last S^T matmul can read a full 256 span
LN_EPS = 1e-5
SCALE = 1.0 / np.sqrt(D * SPAN)

MMDT = BF16
NPDT = ml_dtypes.bfloat16

_cache = {}


def _mm(nc, out, lhsT, rhs, **kw):
    nc.tensor.matmul(out, lhsT, rhs, **kw)


def _build():
    nc = bacc.Bacc("TRN2", target_bir_lowering=False, debug=False,
                   enable_asserts=False, num_devices=8)
    xT = nc.dram_tensor("xT", [D, T], MMDT, kind="ExternalInput").ap()
    W = nc.dram_tensor("W", [12, 128, NCHK * 128], MMDT, kind="ExternalInput").ap()
    WVA = nc.dram_tensor("WVA", [NCHK, 128, D + 4], MMDT, kind="ExternalInput").ap()
    BQK = nc.dram_tensor("BQK", [128, 12], F32, kind="ExternalInput").ap()
    BV = nc.dram_tensor("BV", [128, D + 4], F32, kind="ExternalInput").ap()
    MSK = nc.dram_tensor("MSK", [128, 256], F32, kind="ExternalInput").ap()
    OUT = nc.dram_tensor("out", [T, D], MMDT, kind="ExternalOutput").ap()

    with tile.TileContext(nc) as tc:
        xT_r = xT.rearrange("(c p) t -> p c t", p=128)
        with tc.tile_pool(name="const", bufs=1) as cp, \
             tc.tile_pool(name="xt", bufs=2) as xp, \
             tc.tile_pool(name="kt", bufs=2) as kp, \
             tc.tile_pool(name="vt", bufs=6) as vp, \
             tc.tile_pool(name="st", bufs=3) as stp, \
             tc.tile_pool(name="outp", bufs=3) as outp, \
             tc.tile_pool(name="scr", bufs=2) as scrp, \
             tc.tile_pool(name="stat", bufs=24) as sp, \
             tc.tile_pool(name="pp", bufs=2, space="PSUM") as pp, \
             tc.tile_pool(name="sps", bufs=2, space="PSUM") as sps, \
             tc.tile_pool(name="ops", bufs=4, space="PSUM") as ops:

            # ---- prefetch first x chunk (sync queue; weights go on scalar) ----
            xt0 = xp.tile([128, NCHK, 512], MMDT, tag="xt", name="xt0")
            for c in range(NCHK):
                nc.sync.dma_start(xt0[:, c, :], xT_r[:, c, 0:512])
            # ---- weights + consts in first-use order ----
            wqk = []
            for e in range(12):
                w = cp.tile([128, NCHK, 128], MMDT, tag=f"wqk{e}", name="wqk")
                wqk.append(w)
            nc.scalar.dma_start(wqk[0][:], W[0].rearrange("p (c q) -> p c q", c=NCHK))
            bqk = cp.tile([128, 12], F32, tag="bqk")
            nc.scalar.dma_start(bqk[:], BQK)
            msk = cp.tile([128, 256], F32, tag="msk")
            nc.scalar.dma_start(msk[:], MSK)
            for e in range(1, 12):
                nc.scalar.dma_start(
                    wqk[e][:], W[e].rearrange("p (c q) -> p c q", c=NCHK))
            bv = cp.tile([128, D + 4], F32, tag="bv")
            nc.scalar.dma_start(bv[:], BV)
            wv = []
            for c in range(NCHK):
                w = cp.tile([128, D + 4], MMDT, tag=f"wv{c}")
                nc.scalar.dma_start(w[:], WVA[c])
                wv.append(w)
            eps = cp.tile([128, 1], F32, tag="eps")
            nc.vector.memset(eps[:], LN_EPS)

            # persistent q^T, padded with zeros past T (no input dependency!)
            qt = []
            for c in range(NCHK):
                q = cp.tile([128, TPAD], MMDT, tag=f"qt{c}")
                nc.vector.memset(q[:, T:TPAD], 0.0)
                qt.append(q)

            kt_tiles = {}
            v_tiles = {}
            o_tiles = {}
            st_tiles = {}

            def proj_q(m, xt):
                """q^T slices for tokens [512m, 512m+512): e on partitions."""
                for e in range(6):
                    ps = pp.tile([128, 512], F32, tag="proj")
                    for c in range(NCHK):
                        _mm(nc, ps[:], wqk[e][:, c, :],
                            xt[:, c, :], start=(c == 0), stop=(c == NCHK - 1))
                    nc.vector.tensor_scalar_add(
                        qt[e][:, 512 * m:512 * (m + 1)], ps[:], bqk[:, e:e + 1])

            def proj_k(m, xt):
                # k^T: one tile per 512-chunk [128, 6, 512]
                kt_m = kp.tile([128, NCHK, 512], MMDT, tag="kt", name="ktm")
                kt_tiles[m] = kt_m
                for e in range(6):
                    ps = pp.tile([128, 512], F32, tag="proj")
                    for c in range(NCHK):
                        _mm(nc, ps[:], wqk[6 + e][:, c, :],
                            xt[:, c, :], start=(c == 0), stop=(c == NCHK - 1))
                    nc.scalar.activation(kt_m[:, e, :], ps[:], AF.Identity,
                                         bias=bqk[:, 6 + e:7 + e])

            def proj_v(m, xt):
                # v natural (+ aug row-sum col), per 128-token quarter
                for h in range(4):
                    psA = pp.tile([128, 384], F32, tag="proj")
                    psB = pp.tile([128, 388], F32, tag="proj")
                    for c in range(NCHK):
                        _mm(nc, psA[:], xt[:, c, 128 * h:128 * (h + 1)],
                            wv[c][:, 0:384], start=(c == 0), stop=(c == NCHK - 1))
                    for c in range(NCHK):
                        _mm(nc, psB[:], xt[:, c, 128 * h:128 * (h + 1)],
                            wv[c][:, 384:772], start=(c == 0), stop=(c == NCHK - 1))
                    vt = vp.tile([128, D + 4], MMDT, tag="v")
                    nc.vector.tensor_tensor(vt[:, 0:384], psA[:], bv[:, 0:384],
                                            op=ALU.add)
                    nc.vector.tensor_tensor(vt[:, 384:772], psB[:], bv[:, 384:772],
                                            op=ALU.add)
                    v_tiles[4 * m + h] = vt

            ln_state = {}

            def ln_stats(kb, early_release=False):
                """PSUM-reading stats: squares (Act) + mean/bias (DVE).
                early_release snapshots oa/ob to SBUF so the o PSUM banks
                free promptly (used where a later av waits on these slots)."""
                oa, ob = o_tiles.pop(kb)
                neg_mu = sp.tile([128, 1], F32, tag="stat")
                nc.vector.tensor_scalar_mul(neg_mu[:], ob[:, 384:385], -1.0 / D)
                # bias_e = -mu^2  (eps is folded into sum2e below)
                bias_e = sp.tile([128, 1], F32, tag="stat")
                nc.vector.tensor_scalar(bias_e[:], neg_mu[:], neg_mu[:], -1.0,
                                        op0=ALU.mult, op1=ALU.mult)
                ssqa = sp.tile([128, 1], F32, tag="stat")
                ssqb = sp.tile([128, 1], F32, tag="stat")
                sum2e = sp.tile([128, 1], F32, tag="stat")
                scr = scrp.tile([128, 384], F32, tag="scr")
                nc.scalar.activation(scr[:], oa[:, 0:384], AF.Square,
                                     accum_out=ssqa[:])
                scr2 = scrp.tile([128, 384], F32, tag="scr")
                nc.scalar.activation(scr2[:], ob[:, 0:384], AF.Square,
                                     accum_out=ssqb[:])
                nc.vector.tensor_scalar(sum2e[:], ssqa[:], ssqb[:],
                                        float(LN_EPS * D),
                                        op0=ALU.add, op1=ALU.add)
                if early_release:
                    ca = scrp.tile([128, 384], MMDT, tag="ca", bufs=2)
                    nc.vector.tensor_scalar_mul(ca[:], oa[:, 0:384], 1.0)
                    cb = scrp.tile([128, 384], MMDT, tag="cb", bufs=2)
                    nc.vector.tensor_scalar_mul(cb[:], ob[:, 0:384], 1.0)
                    ln_state[kb] = (ca, cb, neg_mu, bias_e, sum2e)
                else:
                    ln_state[kb] = (oa, ob, neg_mu, bias_e, sum2e)

            def ln_norm(kb):
                """sqrt -> rstd -> concurrent normalizes (DVE a, Act b)."""
                sa, sb, neg_mu, bias_e, sum2e = ln_state.pop(kb)
                std = sp.tile([128, 1], F32, tag="stat")
                nc.scalar.activation(std[:], sum2e[:], AF.Sqrt, bias=bias_e[:],
                                     scale=1.0 / D)
                rstd = sp.tile([128, 1], F32, tag="stat")
                nc.vector.reciprocal(rstd[:], std[:])
                osb = outp.tile([128, D], MMDT, tag="out")
                nc.vector.tensor_scalar(osb[:, 0:384], sa[:, 0:384],
                                        neg_mu[:], rstd[:],
                                        op0=ALU.add, op1=ALU.mult)
                nc.vector.tensor_scalar(osb[:, 384:768], sb[:, 0:384],
                                        neg_mu[:], rstd[:],
                                        op0=ALU.add, op1=ALU.mult)
                if kb == NB - 1:
                    # last block: split the store so the first half goes out
                    # as soon as the DVE-normalized half is ready
                    nc.sync.dma_start(OUT[128 * kb:128 * (kb + 1), 0:384],
                                      osb[:, 0:384])
                    nc.sync.dma_start(OUT[128 * kb:128 * (kb + 1), 384:768],
                                      osb[:, 384:768])
                else:
                    nc.sync.dma_start(OUT[128 * kb:128 * (kb + 1), :], osb[:])

            def ln_store(kb):
                ln_stats(kb)
                ln_norm(kb)

            def score(kb):
                # S^T for key block kb vs queries [128kb, 128kb+256)
                # (last block: only its own 128 queries exist)
                w = 128 if kb == NB - 1 else 256
                st_ps = sps.tile([128, 256], F32, tag="st")
                ktile = kt_tiles[kb // 4]
                koff = 128 * (kb % 4)
                for c in range(NCHK):
                    _mm(nc, st_ps[:, 0:w], ktile[:, c, koff:koff + 128],
                        qt[c][:, 128 * kb:128 * kb + w],
                        start=(c == 0), stop=(c == NCHK - 1))
                st_sb = stp.tile([128, 256], MMDT, tag="stsb")
                nc.vector.tensor_tensor(st_sb[:, 0:w], st_ps[:, 0:w],
                                        msk[:, 0:w], op=ALU.mult)
                st_tiles[kb] = st_sb

            def av(kb):
                st_sb = st_tiles.pop(kb)
                vt = v_tiles.pop(kb)
                if kb == 0:
                    o_tiles[0] = (ops.tile([128, 384], F32, tag="o", name="o0a"),
                                  ops.tile([128, 388], F32, tag="o", name="o0b"))
                oa, ob = o_tiles[kb]
                _mm(nc, oa[:], st_sb[:, 0:128], vt[:, 0:384],
                    start=(kb == 0), stop=True, skip_group_check=True)
                _mm(nc, ob[:], st_sb[:, 0:128], vt[:, 384:772],
                    start=(kb == 0), stop=True, skip_group_check=True)
                if kb < NB - 1:
                    # endgame (no proj left): borrow the idle pp banks for
                    # o(14) so the last av blocks never wait on ln ladders
                    pool = pp if kb + 1 == 14 else ops
                    tag = "proj" if pool is pp else "o"
                    na = pool.tile([128, 384], F32, tag=tag, name="ona")
                    nb_ = pool.tile([128, 388], F32, tag=tag, name="onb")
                    o_tiles[kb + 1] = (na, nb_)
                    _mm(nc, na[:], st_sb[:, 128:256], vt[:, 0:384],
                        start=True, stop=False, skip_group_check=True)
                    _mm(nc, nb_[:], st_sb[:, 128:256], vt[:, 384:772],
                        start=True, stop=False, skip_group_check=True)

            proj_q(0, xt0)
            proj_k(0, xt0)
            proj_v(0, xt0)
            score(0)
            xts = {0: xt0}

            def prefetch_x(mm):
                xt = xp.tile([128, NCHK, 512], MMDT, tag="xt")
                for c in range(NCHK):
                    nc.sync.dma_start(
                        xt[:, c, :], xT_r[:, c, 512 * mm:512 * (mm + 1)])
                xts[mm] = xt

            prefetch_x(1)
            for m in range(NM - 1):
                # proj sub-phases interleave with attention so the PE covers
                # the LN ladder; score(4m+3) needs proj_q(m+1)'s qt columns,
                # score(4m+4) needs proj_k(m+1)'s k^T. Each ln ladder is
                # emitted AFTER the following proj phase so the proj PSUM
                # drains sit ahead of it in the engine queues.
                score(4 * m + 1)
                av(4 * m)
                proj_q(m + 1, xts[m + 1])
                if m + 2 < NM:
                    prefetch_x(m + 2)
                ln_store(4 * m)
                score(4 * m + 2)
                av(4 * m + 1)
                proj_k(m + 1, xts[m + 1])
                ln_store(4 * m + 1)
                score(4 * m + 3)
                av(4 * m + 2)
                proj_v(m + 1, xts[m + 1])
                ln_store(4 * m + 2)
                score(4 * m + 4)
                av(4 * m + 3)
                ln_store(4 * m + 3)
            # endgame (m=3): no proj left to hide ladders behind. Pipeline
            # stats/norm two-deep so the scalar queue never head-of-line
            # blocks at sqrt, and the last av mms run back-to-back.
            score(13)
            av(12)
            ln_stats(12, early_release=True)
            score(14)
            av(13)           # o(14) -> pp, no ladder wait
            ln_stats(13)
            ln_norm(12)
            score(15)
            av(14)           # o(15) -> ops slots freed by copies(12)
            ln_stats(14)
            ln_norm(13)
            av(15)
            ln_stats(15)
            ln_norm(14)
            ln_norm(15)

    nc.compile()
    return nc


def _prepare_common(W_qkv, b_qkv):
    Wfull = np.ascontiguousarray(W_qkv, dtype=np.float32)
    W = np.empty((12, 128, NCHK * 128), dtype=np.float32)
    for e in range(12):
        for c in range(NCHK):
            W[e, :, 128 * c:128 * (c + 1)] = \
                Wfull[128 * c:128 * (c + 1), 128 * e:128 * (e + 1)]
    wv = Wfull[:, 1536:2304]
    WVA = np.zeros((NCHK, 128, D + 4), dtype=np.float32)
    for c in range(NCHK):
        blk = wv[128 * c:128 * (c + 1)]
        WVA[c, :, 0:D] = blk
        WVA[c, :, D] = blk.sum(axis=1)
    BQK = np.ascontiguousarray(
        b_qkv[0:1536].reshape(12, 128).T, dtype=np.float32)
    bva = np.zeros(D + 4, dtype=np.float32)
    bva[0:D] = b_qkv[1536:2304]
    bva[D] = b_qkv[1536:2304].sum()
    BV = np.ascontiguousarray(np.broadcast_to(bva, (128, D + 4)))
    j = np.arange(128)[:, None]
    i = np.arange(256)[None, :]
    MSK = np.where((i - j >= 0) & (i - j < SPAN), SCALE, 0.0).astype(np.float32)
    return W.astype(NPDT), WVA.astype(NPDT), BQK, BV, MSK


def run(inputs, trace=False):
    x = np.asarray(inputs["x"], dtype=np.float32)
    W_qkv = np.asarray(inputs["W_qkv"], dtype=np.float32)
    b_qkv = np.asarray(inputs["b_qkv"], dtype=np.float32)
    if "nc" not in _cache:
        _cache["nc"] = _build()
    nc = _cache["nc"]
    W, WVA, BQK, BV, MSK = _prepare_common(W_qkv, b_qkv)
    xT = np.ascontiguousarray(x.transpose(0, 2, 1)).astype(NPDT)  # [B, D, T]
    in_maps = [
        {"xT": xT[b], "W": W, "WVA": WVA, "BQK": BQK, "BV": BV, "MSK": MSK}
        for b in range(B)
    ]
    res = bass_utils.run_bass_kernel_spmd(
        nc, in_maps, core_ids=list(range(B)), trace=trace)
    return res


def kernel(x, W_qkv, b_qkv, ln_w, ln_b):
    res = run({"x": x, "W_qkv": W_qkv, "b_qkv": b_qkv})
    out = np.stack([np.asarray(res.results[b]["out"], dtype=np.float32)
                    for b in range(B)])
    ln_w = np.asarray(ln_w, dtype=np.float32)
    ln_b = np.asarray(ln_b, dtype=np.float32)
    if not (np.all(ln_w == 1.0) and np.all(ln_b == 0.0)):
        out = out * ln_w + ln_b
    return out


# revision 37
# speedup vs baseline: 1.0558x; 1.0075x over previous
"""Trainium2 Bass kernel for nn_LocalAttentionParallel.

Reference computation (per batch element b):
    qkv = x @ W_qkv + b_qkv ; q,k,v = split(qkv)
    scores = (q @ k^T) * scale, masked to causal sliding window of width 128
    out = LayerNorm(scores @ v) * ln_w + ln_b          (no softmax!)

Sharding: data-parallel over batch B=8 across 8 NeuronCores (1 element each).
Weights replicated. ln_w/ln_b affine applied on host (free; device returns the
normalized tensor).

Device algorithm per core (T=2048, D=768, span=128):
  - 16 key blocks of 128 tokens. Query block t needs keys from blocks t-1, t.
  - All GEMMs in bf16 (1 cycle/row on the PE, same as fp32r, but half the
    HBM/SBUF traffic so weights arrive before the PE needs them). PSUM
    accumulation is fp32; LN statistics are fp32.
  - q^T, k^T (embedding on partitions) come straight out of the projection
    matmuls; v in natural layout. A 769th column of W_v (host-added row sums)
    makes the PE produce row-sums of the attention output for the LN mean.
  - Attention is software-pipelined: S^T(kb+1) is computed while S@V(kb)
    runs, hiding the mask-multiply latency between them.
"""

import numpy as np
import ml_dtypes

import concourse.bass as bass
import concourse.mybir as mybir
import concourse.tile as tile
from concourse import bacc
from concourse import bass_utils

F32 = mybir.dt.float32
BF16 = mybir.dt.bfloat16
AF = mybir.ActivationFunctionType
ALU = mybir.AluOpType

B, T, D = 8, 2048, 768
SPAN = 128
NCHK = 6          # contraction chunks of 128 over D
NB = 16           # 128-token blocks
NM = 4            # 512-token projection chunks
TPAD = T + 128    # q^T padded so the last S^T matmul can read a full 256 span
LN_EPS = 1e-5
SCALE = 1.0 / np.sqrt(D * SPAN)

MMDT = BF16
NPDT = ml_dtypes.bfloat16

_cache = {}


def _mm(nc, out, lhsT, rhs, **kw):
    nc.tensor.matmul(out, lhsT, rhs, **kw)


def _build():
    nc = bacc.Bacc("TRN2", target_bir_lowering=False, debug=False,
                   enable_asserts=False, num_devices=8)
    xT = nc.dram_tensor("xT", [D, T], MMDT, kind="ExternalInput").ap()
    W = nc.dram_tensor("W", [12, 128, NCHK * 128], MMDT, kind="ExternalInput").ap()
    WVA = nc.dram_tensor("WVA", [NCHK, 128, D + 4], MMDT, kind="ExternalInput").ap()
    BQK = nc.dram_tensor("BQK", [128, 12], F32, kind="ExternalInput").ap()
    BV = nc.dram_tensor("BV", [128, D + 4], F32, kind="ExternalInput").ap()
    MSK = nc.dram_tensor("MSK", [128, 256], F32, kind="ExternalInput").ap()
    OUT = nc.dram_tensor("out", [T, D], MMDT, kind="ExternalOutput").ap()

    with tile.TileContext(nc) as tc:
        xT_r = xT.rearrange("(c p) t -> p c t", p=128)
        with tc.tile_pool(name="const", bufs=1) as cp, \
             tc.tile_pool(name="xt", bufs=2) as xp, \
             tc.tile_pool(name="kt", bufs=2) as kp, \
             tc.tile_pool(name="vt", bufs=6) as vp, \
             tc.tile_pool(name="st", bufs=3) as stp, \
             tc.tile_pool(name="outp", bufs=3) as outp, \
             tc.tile_pool(name="scr", bufs=2) as scrp, \
             tc.tile_pool(name="stat", bufs=24) as sp, \
             tc.tile_pool(name="pp", bufs=2, space="PSUM") as pp, \
             tc.tile_pool(name="sps", bufs=2, space="PSUM") as sps, \
             tc.tile_pool(name="ops", bufs=4, space="PSUM") as ops:

            # ---- prefetch first x chunk (sync queue; weights go on scalar) ----
            xt0 = xp.tile([128, NCHK, 512], MMDT, tag="xt", name="xt0")
            for c in range(NCHK):
                nc.sync.dma_start(xt0[:, c, :], xT_r[:, c, 0:512])
            # ---- weights + consts in first-use order ----
            wqk = []
            for e in range(12):
                w = cp.tile([128, NCHK, 128], MMDT, tag=f"wqk{e}", name="wqk")
                wqk.append(w)
            nc.scalar.dma_start(wqk[0][:], W[0].rearrange("p (c q) -> p c q", c=NCHK))
            bqk = cp.tile([128, 12], F32, tag="bqk")
            nc.scalar.dma_start(bqk[:], BQK)
            msk = cp.tile([128, 256], F32, tag="msk")
            nc.scalar.dma_start(msk[:], MSK)
            for e in range(1, 12):
                nc.scalar.dma_start(
                    wqk[e][:], W[e].rearrange("p (c q) -> p c q", c=NCHK))
            bv = cp.tile([128, D + 4], F32, tag="bv")
            nc.scalar.dma_start(bv[:], BV)
            wv = []
            for c in range(NCHK):
                w = cp.tile([128, D + 4], MMDT, tag=f"wv{c}")
                nc.scalar.dma_start(w[:], WVA[c])
                wv.append(w)
            eps = cp.tile([128, 1], F32, tag="eps")
            nc.vector.memset(eps[:], LN_EPS)

            # persistent q^T, padded with zeros past T (no input dependency!)
            qt = []
            for c in range(NCHK):
                q = cp.tile([128, TPAD], MMDT, tag=f"qt{c}")
                nc.vector.memset(q[:, T:TPAD], 0.0)
                qt.append(q)

            kt_tiles = {}
            v_tiles = {}
            o_tiles = {}
            st_tiles = {}

            def proj_q(m, xt):
                """q^T slices for tokens [512m, 512m+512): e on partitions."""
                for e in range(6):
                    ps = pp.tile([128, 512], F32, tag="proj")
                    for c in range(NCHK):
                        _mm(nc, ps[:], wqk[e][:, c, :],
                            xt[:, c, :], start=(c == 0), stop=(c == NCHK - 1))
                    nc.vector.tensor_scalar_add(
                        qt[e][:, 512 * m:512 * (m + 1)], ps[:], bqk[:, e:e + 1])

            def proj_k(m, xt):
                # k^T: one tile per 512-chunk [128, 6, 512]
                kt_m = kp.tile([128, NCHK, 512], MMDT, tag="kt", name="ktm")
                kt_tiles[m] = kt_m
                for e in range(6):
                    ps = pp.tile([128, 512], F32, tag="proj")
                    for c in range(NCHK):
                        _mm(nc, ps[:], wqk[6 + e][:, c, :],
                            xt[:, c, :], start=(c == 0), stop=(c == NCHK - 1))
                    nc.scalar.activation(kt_m[:, e, :], ps[:], AF.Identity,
                                         bias=bqk[:, 6 + e:7 + e])

            def proj_v(m, xt):
                # v natural (+ aug row-sum col), per 128-token quarter
                for h in range(4):
                    psA = pp.tile([128, 384], F32, tag="proj")
                    psB = pp.tile([128, 388], F32, tag="proj")
                    for c in range(NCHK):
                        _mm(nc, psA[:], xt[:, c, 128 * h:128 * (h + 1)],
                            wv[c][:, 0:384], start=(c == 0), stop=(c == NCHK - 1))
                    for c in range(NCHK):
                        _mm(nc, psB[:], xt[:, c, 128 * h:128 * (h + 1)],
                            wv[c][:, 384:772], start=(c == 0), stop=(c == NCHK - 1))
                    vt = vp.tile([128, D + 4], MMDT, tag="v")
                    nc.vector.tensor_tensor(vt[:, 0:384], psA[:], bv[:, 0:384],
                                            op=ALU.add)
                    nc.vector.tensor_tensor(vt[:, 384:772], psB[:], bv[:, 384:772],
                                            op=ALU.add)
                    v_tiles[4 * m + h] = vt

            ln_state = {}

            def ln_stats(kb, early_release=False):
                """PSUM-reading stats: squares (Act) + mean/bias (DVE).
                early_release snapshots oa/ob to SBUF so the o PSUM banks
                free promptly (used where a later av waits on these slots)."""
                oa, ob = o_tiles.pop(kb)
                neg_mu = sp.tile([128, 1], F32, tag="stat")
                nc.vector.tensor_scalar_mul(neg_mu[:], ob[:, 384:385], -1.0 / D)
                # bias_e = -mu^2  (eps is folded into sum2e below)
                bias_e = sp.tile([128, 1], F32, tag="stat")
                nc.vector.tensor_scalar(bias_e[:], neg_mu[:], neg_mu[:], -1.0,
                                        op0=ALU.mult, op1=ALU.mult)
                ssqa = sp.tile([128, 1], F32, tag="stat")
                ssqb = sp.tile([128, 1], F32, tag="stat")
                sum2e = sp.tile([128, 1], F32, tag="stat")
                scr = scrp.tile([128, 384], F32, tag="scr")
                nc.scalar.activation(scr[:], oa[:, 0:384], AF.Square,
                                     accum_out=ssqa[:])
                scr2 = scrp.tile([128, 384], F32, tag="scr")
                nc.scalar.activation(scr2[:], ob[:, 0:384], AF.Square,
                                     accum_out=ssqb[:])
                nc.vector.tensor_scalar(sum2e[:], ssqa[:], ssqb[:],
                                        float(LN_EPS * D),
                                        op0=ALU.add, op1=ALU.add)
                if early_release:
                    ca = scrp.tile([128, 384], MMDT, tag="ca", bufs=2)
                    nc.vector.tensor_scalar_mul(ca[:], oa[:, 0:384], 1.0)
                    cb = scrp.tile([128, 384], MMDT, tag="cb", bufs=2)
                    nc.vector.tensor_scalar_mul(cb[:], ob[:, 0:384], 1.0)
                    ln_state[kb] = (ca, cb, neg_mu, bias_e, sum2e)
                else:
                    ln_state[kb] = (oa, ob, neg_mu, bias_e, sum2e)

            def ln_norm(kb, parallel_b=False):
                """sqrt -> rstd -> normalizes (both DVE, or DVE+Act when the
                ladder latency itself is the critical path)."""
                sa, sb, neg_mu, bias_e, sum2e = ln_state.pop(kb)
                std = sp.tile([128, 1], F32, tag="stat")
                nc.scalar.activation(std[:], sum2e[:], AF.Sqrt, bias=bias_e[:],
                                     scale=1.0 / D)
                rstd = sp.tile([128, 1], F32, tag="stat")
                nc.vector.reciprocal(rstd[:], std[:])
                osb = outp.tile([128, D], MMDT, tag="out")
                nc.vector.tensor_scalar(osb[:, 0:384], sa[:, 0:384],
                                        neg_mu[:], rstd[:],
                                        op0=ALU.add, op1=ALU.mult)
                if parallel_b:
                    bias_b = sp.tile([128, 1], F32, tag="stat")
                    nc.gpsimd.tensor_scalar_mul(bias_b[:], neg_mu[:], rstd[:])
                    nc.scalar.activation(osb[:, 384:768], sb[:, 0:384],
                                         AF.Identity, bias=bias_b[:],
                                         scale=rstd[:])
                else:
                    nc.vector.tensor_scalar(osb[:, 384:768], sb[:, 0:384],
                                            neg_mu[:], rstd[:],
                                            op0=ALU.add, op1=ALU.mult)
                if kb == NB - 1:
                    # last block: split the store so the first half goes out
                    # as soon as the DVE-normalized half is ready
                    nc.sync.dma_start(OUT[128 * kb:128 * (kb + 1), 0:384],
                                      osb[:, 0:384])
                    nc.sync.dma_start(OUT[128 * kb:128 * (kb + 1), 384:768],
                                      osb[:, 384:768])
                else:
                    nc.sync.dma_start(OUT[128 * kb:128 * (kb + 1), :], osb[:])

            def ln_store(kb):
                ln_stats(kb)
                ln_norm(kb)

            def score(kb):
                # S^T for key block kb vs queries [128kb, 128kb+256)
                # (last block: only its own 128 queries exist)
                w = 128 if kb == NB - 1 else 256
                st_ps = sps.tile([128, 256], F32, tag="st")
                ktile = kt_tiles[kb // 4]
                koff = 128 * (kb % 4)
                for c in range(NCHK):
                    _mm(nc, st_ps[:, 0:w], ktile[:, c, koff:koff + 128],
                        qt[c][:, 128 * kb:128 * kb + w],
                        start=(c == 0), stop=(c == NCHK - 1))
                st_sb = stp.tile([128, 256], MMDT, tag="stsb")
                nc.vector.tensor_tensor(st_sb[:, 0:w], st_ps[:, 0:w],
                                        msk[:, 0:w], op=ALU.mult)
                st_tiles[kb] = st_sb

            def av(kb):
                st_sb = st_tiles.pop(kb)
                vt = v_tiles.pop(kb)
                if kb == 0:
                    o_tiles[0] = (ops.tile([128, 384], F32, tag="o", name="o0a"),
                                  ops.tile([128, 388], F32, tag="o", name="o0b"))
                oa, ob = o_tiles[kb]
                _mm(nc, oa[:], st_sb[:, 0:128], vt[:, 0:384],
                    start=(kb == 0), stop=True, skip_group_check=True)
                _mm(nc, ob[:], st_sb[:, 0:128], vt[:, 384:772],
                    start=(kb == 0), stop=True, skip_group_check=True)
                if kb < NB - 1:
                    # endgame (no proj left): borrow the idle pp banks for
                    # o(14) so the last av blocks never wait on ln ladders
                    pool = pp if kb + 1 == 14 else ops
                    tag = "proj" if pool is pp else "o"
                    na = pool.tile([128, 384], F32, tag=tag, name="ona")
                    nb_ = pool.tile([128, 388], F32, tag=tag, name="onb")
                    o_tiles[kb + 1] = (na, nb_)
                    _mm(nc, na[:], st_sb[:, 128:256], vt[:, 0:384],
                        start=True, stop=False, skip_group_check=True)
                    _mm(nc, nb_[:], st_sb[:, 128:256], vt[:, 384:772],
                        start=True, stop=False, skip_group_check=True)

            proj_q(0, xt0)
            proj_k(0, xt0)
            proj_v(0, xt0)
            score(0)
            xts = {0: xt0}

            def prefetch_x(mm, eng=None):
                xt = xp.tile([128, NCHK, 512], MMDT, tag="xt")
                for c in range(NCHK):
                    (eng or nc.sync).dma_start(
                        xt[:, c, :], xT_r[:, c, 512 * mm:512 * (mm + 1)])
                xts[mm] = xt

            # x(1) rides the scalar queue BEHIND the weights: no bandwidth
            # competition during the startup weight stream
            prefetch_x(1, eng=nc.scalar)
            for m in range(NM - 1):
                # proj sub-phases interleave with attention so the PE covers
                # the LN ladder; score(4m+3) needs proj_q(m+1)'s qt columns,
                # score(4m+4) needs proj_k(m+1)'s k^T. Each ln ladder is
                # emitted AFTER the following proj phase so the proj PSUM
                # drains sit ahead of it in the engine queues.
                score(4 * m + 1)
                av(4 * m)
                proj_q(m + 1, xts[m + 1])
                if m + 2 < NM:
                    prefetch_x(m + 2)
                ln_store(4 * m)
                score(4 * m + 2)
                av(4 * m + 1)
                proj_k(m + 1, xts[m + 1])
                ln_store(4 * m + 1)
                score(4 * m + 3)
                av(4 * m + 2)
                proj_v(m + 1, xts[m + 1])
                ln_store(4 * m + 2)
                score(4 * m + 4)
                av(4 * m + 3)
                ln_store(4 * m + 3)
            # endgame (m=3): no proj left to hide ladders behind. Pipeline
            # stats/norm two-deep so the scalar queue never head-of-line
            # blocks at sqrt, and the last av mms run back-to-back.
            score(13)
            av(12)
            ln_stats(12, early_release=True)
            score(14)
            av(13)           # o(14) -> pp, no ladder wait
            ln_stats(13)
            ln_norm(12)
            score(15)
            av(14)           # o(15) -> ops slots freed by copies(12)
            ln_stats(14)
            ln_norm(13)
            av(15)
            ln_stats(15)
            ln_norm(14, parallel_b=True)
            ln_norm(15, parallel_b=True)

    nc.compile()
    return nc


def _prepare_common(W_qkv, b_qkv):
    Wfull = np.ascontiguousarray(W_qkv, dtype=np.float32)
    W = np.empty((12, 128, NCHK * 128), dtype=np.float32)
    for e in range(12):
        for c in range(NCHK):
            W[e, :, 128 * c:128 * (c + 1)] = \
                Wfull[128 * c:128 * (c + 1), 128 * e:128 * (e + 1)]
    wv = Wfull[:, 1536:2304]
    WVA = np.zeros((NCHK, 128, D + 4), dtype=np.float32)
    for c in range(NCHK):
        blk = wv[128 * c:128 * (c + 1)]
        WVA[c, :, 0:D] = blk
        WVA[c, :, D] = blk.sum(axis=1)
    BQK = np.ascontiguousarray(
        b_qkv[0:1536].reshape(12, 128).T, dtype=np.float32)
    bva = np.zeros(D + 4, dtype=np.float32)
    bva[0:D] = b_qkv[1536:2304]
    bva[D] = b_qkv[1536:2304].sum()
    BV = np.ascontiguousarray(np.broadcast_to(bva, (128, D + 4)))
    j = np.arange(128)[:, None]
    i = np.arange(256)[None, :]
    MSK = np.where((i - j >= 0) & (i - j < SPAN), SCALE, 0.0).astype(np.float32)
    return W.astype(NPDT), WVA.astype(NPDT), BQK, BV, MSK


def run(inputs, trace=False):
    x = np.asarray(inputs["x"], dtype=np.float32)
    W_qkv = np.asarray(inputs["W_qkv"], dtype=np.float32)
    b_qkv = np.asarray(inputs["b_qkv"], dtype=np.float32)
    if "nc" not in _cache:
        _cache["nc"] = _build()
    nc = _cache["nc"]
    W, WVA, BQK, BV, MSK = _prepare_common(W_qkv, b_qkv)
    xT = np.ascontiguousarray(x.transpose(0, 2, 1)).astype(NPDT)  # [B, D, T]
    in_maps = [
        {"xT": xT[b], "W": W, "WVA": WVA, "BQK": BQK, "BV": BV, "MSK": MSK}
        for b in range(B)
    ]
    res = bass_utils.run_bass_kernel_spmd(
        nc, in_maps, core_ids=list(range(B)), trace=trace)
    return res


def kernel(x, W_qkv, b_qkv, ln_w, ln_b):
    res = run({"x": x, "W_qkv": W_qkv, "b_qkv": b_qkv})
    out = np.stack([np.asarray(res.results[b]["out"], dtype=np.float32)
                    for b in range(B)])
    ln_w = np.asarray(ln_w, dtype=np.float32)
    ln_b = np.asarray(ln_b, dtype=np.float32)
    if not (np.all(ln_w == 1.0) and np.all(ln_b == 0.0)):
        out = out * ln_w + ln_b
    return out
